# revision 1
# baseline (speedup 1.0000x reference)
"""Trainium2 Bass kernel for nn_ColorFeatureExtractor (per-image KMeans color
extraction). Pure data parallel: image b -> core b. Each core runs 100 Lloyd
iterations entirely on-chip and streams back per-iteration counts + centers
trajectories; the host selects the convergence iteration (faithful to the
reference's global-allclose freeze semantics) and assembles the [B,K,K,4]
output."""
import sys
import numpy as np

for _p in ("/opt/trn_rl_repo", "/root/.axon_site/_ro/trn_rl_repo"):
    if _p not in sys.path:
        sys.path.append(_p)

K = 5
N = 224 * 224          # pixels per image
P = 128                # partitions
F = N // P             # 392 free elems per partition
import os
ITERS = int(os.environ.get("KM_ITERS", "100"))
RTOL, ATOL = 1e-5, 1e-8
OUT_LEN = 500 + 101 * 15   # counts traj + centers traj

_CACHE = {}


def _build_nc():
    import concourse.bass as bass
    import concourse.mybir as mybir
    from concourse import bacc, tile

    f32 = mybir.dt.float32
    Alu = mybir.AluOpType
    Act = mybir.ActivationFunctionType

    nc = bacc.Bacc(None, target_bir_lowering=False)
    xp = nc.dram_tensor("xp", [3, N], f32, kind="ExternalInput")
    cbin = nc.dram_tensor("cbin", [1, 20], f32, kind="ExternalInput")
    outv = nc.dram_tensor("outv", [1, OUT_LEN], f32, kind="ExternalOutput")

    with tile.TileContext(nc) as tc:
        with (
            tc.tile_pool(name="persist", bufs=1) as pp,
            tc.tile_pool(name="sbig", bufs=2) as sb,
            tc.tile_pool(name="scr", bufs=3) as scr,
            tc.tile_pool(name="small", bufs=2) as sm,
            tc.tile_pool(name="psum", bufs=2, space=bass.MemorySpace.PSUM) as ps,
        ):
            # ---- persistent tiles ----
            px = pp.tile([P, F], f32, tag="px")
            py = pp.tile([P, F], f32, tag="py")
            pz = pp.tile([P, F], f32, tag="pz")
            ones_col = pp.tile([P, 1], f32, tag="ones_col")    # matmul lhsT for col-sum
            ones_row = pp.tile([1, P], f32, tag="ones_row")    # matmul lhsT for broadcast
            tot3 = pp.tile([1, 3], f32, tag="tot3")            # sum of px/py/pz
            counts_st = pp.tile([1, 500], f32, tag="counts_st")
            cent_st = pp.tile([1, 101 * 15], f32, tag="cent_st")

            nc.vector.memset(counts_st[:], 0.0)
            nc.vector.memset(cent_st[:], 0.0)
            xap = xp[:].rearrange("c (p f) -> c p f", p=P)
            nc.sync.dma_start(out=px[:], in_=xap[0])
            nc.sync.dma_start(out=py[:], in_=xap[1])
            nc.sync.dma_start(out=pz[:], in_=xap[2])
            cb0 = pp.tile([1, 20], f32, tag="cb0")
            nc.sync.dma_start(out=cb0[:], in_=cbin[:])

            nc.vector.memset(ones_col[:], 1.0)
            nc.vector.memset(ones_row[:], 1.0)

            # pixels = x + 1e-8, vector-owned; gpsimd gets private copies so
            # its loop-body ops never need cross-engine waits (HW structs have
            # very few sync-wait slots)
            nc.vector.tensor_scalar(px[:], px[:], 1e-8, None, Alu.add)
            nc.vector.tensor_scalar(py[:], py[:], 1e-8, None, Alu.add)
            nc.vector.tensor_scalar(pz[:], pz[:], 1e-8, None, Alu.add)


            planes0 = (px, py, pz)
            # totals: [1,3] = sum of each plane
            totc = pp.tile([P, 3], f32, tag="totc")
            nc.vector.tensor_reduce(totc[:, 0:1], px[:], mybir.AxisListType.X, Alu.add)
            nc.vector.tensor_reduce(totc[:, 1:2], py[:], mybir.AxisListType.X, Alu.add)
            nc.vector.tensor_reduce(totc[:, 2:3], pz[:], mybir.AxisListType.X, Alu.add)
            tot3_ps = ps.tile([1, 3], f32, tag="tot3ps")
            nc.tensor.matmul(tot3_ps[:], ones_col[:], totc[:], start=True, stop=True)
            nc.vector.tensor_copy(tot3[:], tot3_ps[:])

            # interleaved pixel tile [p, f*3] = (x,y,z) per pixel, for the
            # one-TT-per-cluster product in phase 3
            pint = pp.tile([P, 3 * F], f32, tag="pint")
            for d in range(3):
                nc.vector.tensor_copy(
                    pint[:].rearrange("p (f d) -> p d f", d=3)[:, d], planes0[d][:]
                )

            # initial centers into trajectory + initial rep broadcast
            nc.scalar.copy(cent_st[0:1, 0:15], cb0[0:1, 0:15])

            cb0v = pp.tile([1, 20], f32, tag="cb0v")
            nc.vector.tensor_copy(cb0v[:], cb0[:])
            rep_ps0 = ps.tile([P, 20], f32, tag="repps")
            nc.tensor.matmul(rep_ps0[:], ones_row[:], cb0v[:], start=True, stop=True)
            rep = sb.tile([P, 20], f32, tag="rep")
            nc.vector.tensor_copy(rep[:], rep_ps0[:])

            for t in range(1, ITERS + 1):
                # ---------- phase 1: scores s_k = px*cx + py*cy + pz*cz + b ----------
                s_tiles = []
                for k in range(5):
                    u = scr.tile([P, F], f32, tag=f"u{k}")
                    # u = px*cx_k + b_k (ACT free affine with AP scale/bias)
                    nc.scalar.activation(
                        u[:], px[:], Act.Identity,
                        bias=rep[:, 15 + k : 16 + k], scale=rep[:, 3 * k : 3 * k + 1],
                    )
                    v = scr.tile([P, F], f32, tag=f"v{k}")
                    s = sb.tile([P, F], f32, tag=f"s{k}")
                    nc.vector.scalar_tensor_tensor(
                        v[:], py[:], rep[:, 3 * k + 1 : 3 * k + 2], u[:], Alu.mult, Alu.add
                    )
                    nc.vector.scalar_tensor_tensor(
                        s[:], pz[:], rep[:, 3 * k + 2 : 3 * k + 3], v[:], Alu.mult, Alu.add
                    )
                    s_tiles.append(s)

                # ---------- phase 2: m = max_k s_k ----------
                m01 = scr.tile([P, F], f32, tag="m01")
                m23 = scr.tile([P, F], f32, tag="m23")
                m = sb.tile([P, F], f32, tag="m")
                nc.vector.tensor_tensor(m01[:], s_tiles[0][:], s_tiles[1][:], Alu.max)
                nc.vector.tensor_tensor(m23[:], s_tiles[2][:], s_tiles[3][:], Alu.max)
                nc.vector.tensor_tensor(m01[:], m01[:], s_tiles[4][:], Alu.max)
                nc.vector.tensor_tensor(m[:], m01[:], m23[:], Alu.max)

                # ---------- phase 3: masks, counts, sums ----------
                acc_d = sb.tile([P, 16], f32, tag="acc_d")   # cnt0..3, S0xyz..S3xyz
                junk_a = scr.tile([P, F], f32, tag="junk_a")
                for k in range(4):
                    mask = scr.tile([P, F], f32, tag=f"mask{k}")
                    nc.vector.tensor_tensor(mask[:], s_tiles[k][:], m[:], Alu.is_equal)
                    # counts via ACT fused row-accumulate
                    nc.scalar.activation(
                        junk_a[:], mask[:], Act.Identity,
                        accum_out=acc_d[:, k : k + 1],
                    )
                    # products for all 3 channels in one strided TT
                    prod3 = scr.tile([P, 3 * F], f32, tag=f"prod{k}")
                    nc.vector.tensor_tensor(
                        prod3[:].rearrange("p (d f) -> p f d", f=F),
                        mask[:].rearrange("p (f o) -> p f o", o=1).broadcast_to((P, F, 3)),
                        pint[:].rearrange("p (f d) -> p f d", d=3),
                        Alu.mult,
                    )
                    for d in range(3):
                        nc.scalar.activation(
                            junk_a[:], prod3[:, d * F : (d + 1) * F], Act.Identity,
                            accum_out=acc_d[:, 4 + 3 * k + d : 5 + 3 * k + d],
                        )

                # ---------- tail: totals -> new centers ----------
                tot = ps.tile([1, 16], f32, tag="tot")
                nc.tensor.matmul(tot[:], ones_col[:], acc_d[:], start=True, stop=True)
                tots = sm.tile([1, 16], f32, tag="tots")
                nc.vector.tensor_copy(tots[:], tot[:])

                cnts = sm.tile([1, 5], f32, tag="cnts")
                csum = sm.tile([1, 1], f32, tag="csum")
                nc.vector.tensor_copy(cnts[0:1, 0:4], tots[0:1, 0:4])
                nc.vector.tensor_reduce(csum[:], tots[0:1, 0:4], mybir.AxisListType.X, Alu.add)
                nc.vector.tensor_scalar(cnts[0:1, 4:5], csum[:], -1.0, float(N), Alu.mult, Alu.add)

                S15 = sm.tile([1, 15], f32, tag="S15")
                s4p = sm.tile([1, 3], f32, tag="s4p")
                nc.vector.tensor_copy(S15[0:1, 0:12], tots[0:1, 4:16])
                # sum over k of S_kd: view cols 4..16 as [d(stride1,3), k(stride3,4)], reduce X
                nc.vector.tensor_reduce(
                    s4p[:], tots[0:1, 4:16].rearrange("p (k d) -> p d k", d=3),
                    mybir.AxisListType.X, Alu.add,
                )
                nc.vector.tensor_tensor(S15[0:1, 12:15], tot3[:], s4p[:], Alu.subtract)

                recip = sm.tile([1, 5], f32, tag="recip")
                nc.vector.reciprocal(recip[:], cnts[:])
                recip15 = sm.tile([1, 15], f32, tag="recip15")
                for d in range(3):
                    nc.scalar.copy(recip15[0:1, d:15:3].rearrange("p (a b) -> p (b a)", b=1), recip[:])

                cb = sm.tile([1, 20], f32, tag="cb")
                nc.vector.tensor_tensor(cb[0:1, 0:15], S15[:], recip15[:], Alu.mult)

                sq = sm.tile([1, 15], f32, tag="sq")
                c2 = sm.tile([1, 5], f32, tag="c2")
                nc.vector.tensor_tensor(sq[:], cb[0:1, 0:15], cb[0:1, 0:15], Alu.mult)
                nc.vector.tensor_reduce(
                    c2[:], sq[:].rearrange("p (k d) -> p k d", d=3),
                    mybir.AxisListType.X, Alu.add,
                )
                nc.vector.tensor_scalar(cb[0:1, 15:20], c2[:], -0.5, 2.0, Alu.mult, Alu.add)

                # store trajectories (ScalarE, off critical path)
                nc.scalar.copy(counts_st[0:1, 5 * (t - 1) : 5 * t], cnts[:])
                nc.scalar.copy(cent_st[0:1, 15 * t : 15 * (t + 1)], cb[0:1, 0:15])

                # broadcast for next iteration
                rep_ps = ps.tile([P, 20], f32, tag="repps")
                nc.tensor.matmul(rep_ps[:], ones_row[:], cb[:], start=True, stop=True)
                rep = sb.tile([P, 20], f32, tag="rep")
                nc.vector.tensor_copy(rep[:], rep_ps[:])
                rep_g = sb.tile([P, 20], f32, tag="rep_g")
                nc.gpsimd.tensor_copy(rep_g[:], rep[:])

            nc.sync.dma_start(out=outv[0:1, 0:500], in_=counts_st[:])
            nc.sync.dma_start(out=outv[0:1, 500:OUT_LEN], in_=cent_st[:])
    nc.compile()
    return nc


def _get_nc():
    if "nc" not in _CACHE:
        _CACHE["nc"] = _build_nc()
    return _CACHE["nc"]


def _host_finalize(counts_all, cent_all):
    """counts_all [B,100,5], cent_all [B,101,15] -> [B,K,K,4] per reference."""
    B = counts_all.shape[0]
    prev = cent_all[:, :-1, :]   # centers entering iter t (t=1..100)
    new = cent_all[:, 1:, :]     # new_centers at iter t
    with np.errstate(invalid="ignore"):
        ok = np.abs(prev - new) <= np.float32(ATOL) + np.float32(RTOL) * np.abs(new)
    conv_t = np.all(ok, axis=(0, 2))          # [100] global allclose per iter
    idx = np.nonzero(conv_t)[0]
    T = int(idx[0]) + 1 if len(idx) else ITERS + 1
    L = min(T, ITERS)
    centers = cent_all[:, T - 1].reshape(B, K, 3)
    percentages = counts_all[:, L - 1] / np.float32(N)
    centers = np.clip(centers, 0.0, 1.0)
    percentages = np.clip(percentages, 0.0, 1.0)
    color_info = np.concatenate([centers, percentages[..., None]], axis=2).astype(np.float32)
    color_info = np.nan_to_num(color_info, nan=0.0, posinf=1.0, neginf=0.0)
    sort_idx = np.argsort(-color_info[:, :, 3], axis=1, kind="stable")
    return color_info[sort_idx]


def _make_inputs(x, init_idx):
    B = x.shape[0]
    x = np.ascontiguousarray(np.asarray(x, dtype=np.float32))
    init_idx = np.asarray(init_idx).astype(np.int64)
    hh, ww = init_idx // 224, init_idx % 224
    in_maps = []
    for b in range(B):
        c0 = (x[b, :, hh, ww] + np.float32(1e-8)).astype(np.float32)  # [5,3]
        cb0 = np.zeros((1, 20), np.float32)
        cb0[0, :15] = c0.reshape(15)
        c2 = (c0 * c0).sum(axis=1, dtype=np.float32)
        cb0[0, 15:20] = np.float32(2.0) - np.float32(0.5) * c2
        in_maps.append({"xp": x[b].reshape(3, N), "cbin": cb0})
    return in_maps


def kernel(x, init_idx):
    from concourse.bass_utils import run_bass_kernel_spmd

    nc = _get_nc()
    in_maps = _make_inputs(x, init_idx)
    res = run_bass_kernel_spmd(nc, in_maps, list(range(8)))
    outs = [np.asarray(r["outv"]).reshape(OUT_LEN) for r in res.results]
    counts_all = np.stack([o[0:500].reshape(100, 5) for o in outs])
    cent_all = np.stack([o[500:OUT_LEN].reshape(101, 15) for o in outs])
    return _host_finalize(counts_all, cent_all)



# revision 2
# speedup vs baseline: 1.0992x; 1.0992x over previous
"""Trainium2 Bass kernel v3 for nn_ColorFeatureExtractor (per-image KMeans).

Data parallel: image b -> core b; 100 Lloyd iterations on-chip; trajectories
streamed back; host picks convergence iteration (same contract as baseline).

v3 vs v2: no gpsimd (it contends with DVE for SBUF ports). fp16 masks and
fp16 channel-major pixel copy; products for all 4 clusters batched into one
2x-rate fp16 TT per channel; sums accumulated on ScalarE ACT (except a few
routed to DVE STT+acc for balance). Counts ride the mask STTs' accumulators
(exact integer sums). Tail on replicated rows.
"""
import os
import sys
import numpy as np

for _p in ("/opt/trn_rl_repo", "/root/.axon_site/_ro/trn_rl_repo"):
    if _p not in sys.path:
        sys.path.append(_p)

K = 5
N = 224 * 224
P = 128
F = N // P             # 392
ITERS = int(os.environ.get("KM_ITERS", "100"))
RTOL, ATOL = 1e-5, 1e-8
OUT_LEN = 500 + 101 * 15

# sums routing: 'S' = ScalarE ACT+acc reading the batched fp16 product,
#               'V' = DVE STT+acc computing product+sum itself (fp16 pixels)
# prod16: batched fp16 products (0 -> fp32 separate TT products, no batching)
_DEF = "sums=SSSVVVVVVVVV;prod16=1"
CFG = os.environ.get("KM_CFG", _DEF)

_CACHE = {}


def _parse_cfg(cfg):
    parts = dict(p.split("=", 1) for p in cfg.split(";") if p)
    sums = parts.get("sums", "S" * 12)
    assert len(sums) == 12 and set(sums) <= set("VS")
    return sums, parts.get("prod16", "1") == "1"


def _build_nc():
    import concourse.bass as bass
    import concourse.mybir as mybir
    from concourse import bacc, tile

    f32 = mybir.dt.float32
    f16 = mybir.dt.float16
    Alu = mybir.AluOpType
    Act = mybir.ActivationFunctionType
    X = mybir.AxisListType.X

    SUMS, PROD16 = _parse_cfg(CFG)
    mdt = f16 if PROD16 else f32

    nc = bacc.Bacc(None, target_bir_lowering=False)
    xp = nc.dram_tensor("xp", [3, N], f32, kind="ExternalInput")
    cbin = nc.dram_tensor("cbin", [1, 20], f32, kind="ExternalInput")
    outv = nc.dram_tensor("outv", [1, OUT_LEN], f32, kind="ExternalOutput")

    with tile.TileContext(nc) as tc:
        with (
            tc.tile_pool(name="persist", bufs=1) as pp,
            tc.tile_pool(name="sco", bufs=2) as sco,
            tc.tile_pool(name="msk", bufs=2) as msk,
            tc.tile_pool(name="jnk", bufs=3) as jnk,
            tc.tile_pool(name="sm", bufs=2) as sm,
            tc.tile_pool(name="ps", bufs=2, space=bass.MemorySpace.PSUM) as ps,
        ):
            # ---- persistent data ----
            pc = pp.tile([P, 3 * F], f32, tag="pc")     # channel-major pixels
            pc16 = pp.tile([P, 3 * F], mdt, tag="pc16")  # product copy
            ones128 = pp.tile([P, 128], f32, tag="ones128")
            onesr = pp.tile([1, 128], f32, tag="onesr")
            tot3 = pp.tile([P, 3], f32, tag="tot3")
            counts_st = pp.tile([1, 500], f32, tag="counts_st")
            cent_st = pp.tile([1, 101 * 15], f32, tag="cent_st")
            cb0 = pp.tile([1, 20], f32, tag="cb0")

            nc.vector.memset(counts_st[:], 0.0)
            nc.vector.memset(cent_st[:], 0.0)
            nc.vector.memset(ones128[:], 1.0)
            nc.vector.memset(onesr[:], 1.0)

            xap = xp[:].rearrange("c (p f) -> c p f", p=P)
            for d in range(3):
                nc.sync.dma_start(out=pc[:, d * F:(d + 1) * F], in_=xap[d])
            nc.sync.dma_start(out=cb0[:], in_=cbin[:])

            px = pc[:, 0:F]
            py = pc[:, F:2 * F]
            pz = pc[:, 2 * F:3 * F]
            for d in range(3):
                nc.vector.tensor_scalar(pc[:, d * F:(d + 1) * F],
                                        pc[:, d * F:(d + 1) * F],
                                        1e-8, None, Alu.add)
            nc.vector.tensor_copy(pc16[:], pc[:])

            # totals per channel from the PRODUCT pixels (consistency for S4)
            totc = pp.tile([P, 3], f32, tag="totc")
            for d in range(3):
                nc.vector.tensor_reduce(totc[:, d:d + 1],
                                        pc16[:, d * F:(d + 1) * F], X, Alu.add)
            tot3_ps = ps.tile([P, 3], f32, tag="tot3ps")
            nc.tensor.matmul(tot3_ps[:], ones128[:], totc[:], start=True, stop=True)
            nc.vector.tensor_copy(tot3[:], tot3_ps[:])

            nc.scalar.copy(cent_st[0:1, 0:15], cb0[0:1, 0:15])
            rep_ps0 = ps.tile([P, 20], f32, tag="repps0")
            nc.tensor.matmul(rep_ps0[:], onesr[:], cb0[:], start=True, stop=True)
            rep = pp.tile([P, 20], f32, tag="rep0")
            nc.vector.tensor_copy(rep[:], rep_ps0[:])

            prev_traj = None
            for t in range(1, ITERS + 1):
                # ---------- phase 1: scores (identical arithmetic to baseline) ----------
                s_tiles = []
                for k in range(5):
                    u = jnk.tile([P, F], f32, tag=f"u{k}")
                    nc.scalar.activation(
                        u[:], px, Act.Identity,
                        bias=rep[:, 15 + k:16 + k], scale=rep[:, 3 * k:3 * k + 1],
                    )
                    v = jnk.tile([P, F], f32, tag=f"v{k}")
                    s = sco.tile([P, F], f32, tag=f"s{k}")
                    nc.vector.scalar_tensor_tensor(
                        v[:], py, rep[:, 3 * k + 1:3 * k + 2], u[:], Alu.mult, Alu.add)
                    nc.vector.scalar_tensor_tensor(
                        s[:], pz, rep[:, 3 * k + 2:3 * k + 3], v[:], Alu.mult, Alu.add)
                    s_tiles.append(s)

                # previous iteration's trajectory snapshots go behind the u's
                # on ScalarE's queue so they don't delay next-iter scores
                if prev_traj is not None:
                    pcn, pcb, pt = prev_traj
                    nc.scalar.copy(counts_st[0:1, 5 * (pt - 1):5 * pt], pcn[0:1, :])
                    nc.scalar.copy(cent_st[0:1, 15 * pt:15 * (pt + 1)], pcb[0:1, 0:15])

                # ---------- phase 2: max (exact) ----------
                m01 = jnk.tile([P, F], f32, tag="m01")
                m23 = jnk.tile([P, F], f32, tag="m23")
                m = sco.tile([P, F], f32, tag="m")
                nc.vector.tensor_tensor(m01[:], s_tiles[0][:], s_tiles[1][:], Alu.max)
                nc.vector.tensor_tensor(m23[:], s_tiles[2][:], s_tiles[3][:], Alu.max)
                nc.vector.tensor_tensor(m01[:], m01[:], s_tiles[4][:], Alu.max)
                nc.vector.tensor_tensor(m[:], m01[:], m23[:], Alu.max)

                # ---------- phase 3: masks+counts, batched products, sums ----------
                acc = sm.tile([P, 16], f32, tag="acc")
                mask4 = msk.tile([P, 4 * F], mdt, tag="mask4")
                for k in range(4):
                    nc.vector.scalar_tensor_tensor(
                        mask4[:, k * F:(k + 1) * F], s_tiles[k][:], 1.0, m[:],
                        Alu.mult, Alu.is_equal, accum_out=acc[:, k:k + 1])

                # counts all-reduce as soon as the 4 mask accums land; the
                # count/reciprocal tail chain then overlaps the sums phase
                tots_c = ps.tile([P, 4], f32, tag="tots_c")
                nc.tensor.matmul(tots_c[:], ones128[:], acc[:, 0:4], start=True, stop=True)
                cnts = sm.tile([P, 5], f32, tag="cnts")
                csum = sm.tile([P, 1], f32, tag="csum")
                nc.vector.tensor_copy(cnts[:, 0:4], tots_c[:, 0:4])
                nc.vector.tensor_reduce(csum[:], tots_c[:, 0:4], X, Alu.add)
                nc.vector.tensor_scalar(cnts[:, 4:5], csum[:], -1.0, float(N),
                                        Alu.mult, Alu.add)
                recip = sm.tile([P, 5], f32, tag="recip")
                nc.vector.reciprocal(recip[:], cnts[:])

                if PROD16:
                    # products only for the S-routed (k,d); batch per channel
                    # when >=3 clusters need it, else per-cluster TTs
                    pr_tiles = {}
                    for d in range(3):
                        ks = [k for k in range(4) if SUMS[3 * k + d] == "S"]
                        if len(ks) >= 3:
                            pr = jnk.tile([P, 4 * F], mdt, tag=f"pr4{d}")
                            nc.vector.tensor_tensor(
                                pr[:].rearrange("p (k f) -> p k f", k=4),
                                mask4[:].rearrange("p (k f) -> p k f", k=4),
                                pc16[:, d * F:(d + 1) * F]
                                    .rearrange("p (o f) -> p o f", o=1)
                                    .broadcast_to((P, 4, F)),
                                Alu.mult)
                            for k in ks:
                                pr_tiles[(k, d)] = pr[:, k * F:(k + 1) * F]
                        else:
                            for k in ks:
                                pr = jnk.tile([P, F], mdt, tag=f"pr{k}{d}")
                                nc.vector.tensor_tensor(
                                    pr[:], mask4[:, k * F:(k + 1) * F],
                                    pc16[:, d * F:(d + 1) * F], Alu.mult)
                                pr_tiles[(k, d)] = pr[:]
                    for k in range(4):
                        for d in range(3):
                            col = acc[:, 4 + 3 * k + d:5 + 3 * k + d]
                            if SUMS[3 * k + d] == "S":
                                ja = jnk.tile([P, F], mdt, tag=f"ja{k}{d}")
                                nc.scalar.activation(
                                    ja[:], pr_tiles[(k, d)],
                                    Act.Identity, accum_out=col)
                            else:
                                j = jnk.tile([P, F], mdt, tag=f"jv{k}{d}")
                                nc.vector.scalar_tensor_tensor(
                                    j[:], pc16[:, d * F:(d + 1) * F], 1.0,
                                    mask4[:, k * F:(k + 1) * F], Alu.mult,
                                    Alu.mult, accum_out=col)
                else:
                    for k in range(4):
                        for d in range(3):
                            col = acc[:, 4 + 3 * k + d:5 + 3 * k + d]
                            pcd = pc[:, d * F:(d + 1) * F]
                            if SUMS[3 * k + d] == "S":
                                pr = jnk.tile([P, F], f32, tag=f"pr{k}{d}")
                                nc.vector.tensor_tensor(
                                    pr[:], mask4[:, k * F:(k + 1) * F], pcd, Alu.mult)
                                ja = jnk.tile([P, F], f32, tag=f"ja{k}{d}")
                                nc.scalar.activation(
                                    ja[:], pr[:], Act.Identity, accum_out=col)
                            else:
                                j = jnk.tile([P, F], f32, tag=f"jv{k}{d}")
                                nc.vector.scalar_tensor_tensor(
                                    j[:], pcd, 1.0, mask4[:, k * F:(k + 1) * F],
                                    Alu.mult, Alu.mult, accum_out=col)

                # ---------- tail (sums all-reduce + center update) ----------
                tots = ps.tile([P, 12], f32, tag="tots")
                nc.tensor.matmul(tots[:], ones128[:], acc[:, 4:16], start=True, stop=True)

                cb = sm.tile([P, 20], f32, tag="cb")
                nc.vector.tensor_tensor(
                    cb[:, 0:12].rearrange("p (k d) -> p k d", d=3),
                    tots[:, 0:12].rearrange("p (k d) -> p k d", d=3),
                    recip[:, 0:4].rearrange("p (k o) -> p k o", o=1).broadcast_to((P, 4, 3)),
                    Alu.mult)
                s4p = sm.tile([P, 3], f32, tag="s4p")
                nc.vector.tensor_reduce(
                    s4p[:], tots[:, 0:12].rearrange("p (k d) -> p d k", d=3), X, Alu.add)
                S4 = sm.tile([P, 3], f32, tag="S4")
                nc.vector.tensor_tensor(S4[:], tot3[:], s4p[:], Alu.subtract)
                nc.vector.tensor_tensor(
                    cb[:, 12:15],
                    S4[:], recip[:, 4:5].broadcast_to((P, 3)), Alu.mult)
                sq = sm.tile([P, 15], f32, tag="sq")
                c2 = sm.tile([P, 5], f32, tag="c2")
                nc.vector.tensor_tensor(sq[:], cb[:, 0:15], cb[:, 0:15], Alu.mult)
                nc.vector.tensor_reduce(
                    c2[:], sq[:].rearrange("p (k d) -> p k d", d=3), X, Alu.add)
                nc.vector.tensor_scalar(cb[:, 15:20], c2[:], -0.5, 2.0,
                                        Alu.mult, Alu.add)

                prev_traj = (cnts, cb, t)
                rep = cb

            # flush final trajectory snapshots
            pcn, pcb, pt = prev_traj
            nc.scalar.copy(counts_st[0:1, 5 * (pt - 1):5 * pt], pcn[0:1, :])
            nc.scalar.copy(cent_st[0:1, 15 * pt:15 * (pt + 1)], pcb[0:1, 0:15])

            nc.sync.dma_start(out=outv[0:1, 0:500], in_=counts_st[:])
            nc.sync.dma_start(out=outv[0:1, 500:OUT_LEN], in_=cent_st[:])
    nc.compile()
    return nc


def _get_nc():
    if "nc" not in _CACHE:
        _CACHE["nc"] = _build_nc()
    return _CACHE["nc"]


def _host_finalize(counts_all, cent_all):
    B = counts_all.shape[0]
    prev = cent_all[:, :-1, :]
    new = cent_all[:, 1:, :]
    with np.errstate(invalid="ignore"):
        ok = np.abs(prev - new) <= np.float32(ATOL) + np.float32(RTOL) * np.abs(new)
    conv_t = np.all(ok, axis=(0, 2))
    idx = np.nonzero(conv_t)[0]
    T = int(idx[0]) + 1 if len(idx) else ITERS + 1
    L = min(T, ITERS)
    centers = cent_all[:, T - 1].reshape(B, K, 3)
    percentages = counts_all[:, L - 1] / np.float32(N)
    centers = np.clip(centers, 0.0, 1.0)
    percentages = np.clip(percentages, 0.0, 1.0)
    color_info = np.concatenate([centers, percentages[..., None]], axis=2).astype(np.float32)
    color_info = np.nan_to_num(color_info, nan=0.0, posinf=1.0, neginf=0.0)
    sort_idx = np.argsort(-color_info[:, :, 3], axis=1, kind="stable")
    return color_info[sort_idx]


def _make_inputs(x, init_idx):
    B = x.shape[0]
    x = np.ascontiguousarray(np.asarray(x, dtype=np.float32))
    init_idx = np.asarray(init_idx).astype(np.int64)
    hh, ww = init_idx // 224, init_idx % 224
    in_maps = []
    for b in range(B):
        c0 = (x[b, :, hh, ww] + np.float32(1e-8)).astype(np.float32)
        cb0 = np.zeros((1, 20), np.float32)
        cb0[0, :15] = c0.reshape(15)
        c2 = (c0 * c0).sum(axis=1, dtype=np.float32)
        cb0[0, 15:20] = np.float32(2.0) - np.float32(0.5) * c2
        in_maps.append({"xp": x[b].reshape(3, N), "cbin": cb0})
    return in_maps


def kernel(x, init_idx):
    from concourse.bass_utils import run_bass_kernel_spmd

    nc = _get_nc()
    in_maps = _make_inputs(x, init_idx)
    res = run_bass_kernel_spmd(nc, in_maps, list(range(8)))
    outs = [np.asarray(r["outv"]).reshape(OUT_LEN) for r in res.results]
    counts_all = np.stack([o[0:500].reshape(100, 5) for o in outs])
    cent_all = np.stack([o[500:OUT_LEN].reshape(101, 15) for o in outs])
    return _host_finalize(counts_all, cent_all)


# revision 3
# speedup vs baseline: 1.1259x; 1.0243x over previous
"""Trainium2 Bass kernel v3 for nn_ColorFeatureExtractor (per-image KMeans).

Data parallel: image b -> core b; 100 Lloyd iterations on-chip; trajectories
streamed back; host picks convergence iteration (same contract as baseline).

v3 vs v2: no gpsimd (it contends with DVE for SBUF ports). fp16 masks and
fp16 channel-major pixel copy; products for all 4 clusters batched into one
2x-rate fp16 TT per channel; sums accumulated on ScalarE ACT (except a few
routed to DVE STT+acc for balance). Counts ride the mask STTs' accumulators
(exact integer sums). Tail on replicated rows.
"""
import os
import sys
import numpy as np

for _p in ("/opt/trn_rl_repo", "/root/.axon_site/_ro/trn_rl_repo"):
    if _p not in sys.path:
        sys.path.append(_p)

K = 5
N = 224 * 224
P = 128
F = N // P             # 392
ITERS = int(os.environ.get("KM_ITERS", "100"))
RTOL, ATOL = 1e-5, 1e-8
OUT_LEN = 500 + 101 * 15

# sums routing: 'S' = ScalarE ACT+acc reading the batched fp16 product,
#               'V' = DVE STT+acc computing product+sum itself (fp16 pixels)
# prod16: batched fp16 products (0 -> fp32 separate TT products, no batching)
_DEF = "sums=SSSSVVSVVSVV;prod16=1"
CFG = os.environ.get("KM_CFG", _DEF)

_CACHE = {}


def _parse_cfg(cfg):
    parts = dict(p.split("=", 1) for p in cfg.split(";") if p)
    sums = parts.get("sums", "S" * 12)
    assert len(sums) == 12 and set(sums) <= set("VS")
    return sums, parts.get("prod16", "1") == "1"


def _build_nc():
    import concourse.bass as bass
    import concourse.mybir as mybir
    from concourse import bacc, tile

    f32 = mybir.dt.float32
    f16 = mybir.dt.float16
    Alu = mybir.AluOpType
    Act = mybir.ActivationFunctionType
    X = mybir.AxisListType.X

    SUMS, PROD16 = _parse_cfg(CFG)
    mdt = f16 if PROD16 else f32

    nc = bacc.Bacc(None, target_bir_lowering=False)
    xp = nc.dram_tensor("xp", [3, N], f32, kind="ExternalInput")
    cbin = nc.dram_tensor("cbin", [1, 20], f32, kind="ExternalInput")
    outv = nc.dram_tensor("outv", [1, OUT_LEN], f32, kind="ExternalOutput")

    with tile.TileContext(nc) as tc:
        with (
            tc.tile_pool(name="persist", bufs=1) as pp,
            tc.tile_pool(name="sco", bufs=2) as sco,
            tc.tile_pool(name="msk", bufs=2) as msk,
            tc.tile_pool(name="jnk", bufs=3) as jnk,
            tc.tile_pool(name="sm", bufs=2) as sm,
            tc.tile_pool(name="ps", bufs=2, space=bass.MemorySpace.PSUM) as ps,
        ):
            # ---- persistent data ----
            pc = pp.tile([P, 3 * F], f32, tag="pc")     # channel-major pixels
            pc16 = pp.tile([P, 3 * F], mdt, tag="pc16")  # product copy
            ones128 = pp.tile([P, 128], f32, tag="ones128")
            onesr = pp.tile([1, 128], f32, tag="onesr")
            tot3 = pp.tile([P, 3], f32, tag="tot3")
            counts_st = pp.tile([1, 500], f32, tag="counts_st")
            cent_st = pp.tile([1, 101 * 15], f32, tag="cent_st")
            cb0 = pp.tile([1, 20], f32, tag="cb0")

            nc.vector.memset(counts_st[:], 0.0)
            nc.vector.memset(cent_st[:], 0.0)
            nc.vector.memset(ones128[:], 1.0)
            nc.vector.memset(onesr[:], 1.0)

            xap = xp[:].rearrange("c (p f) -> c p f", p=P)
            for d in range(3):
                nc.sync.dma_start(out=pc[:, d * F:(d + 1) * F], in_=xap[d])
            nc.sync.dma_start(out=cb0[:], in_=cbin[:])

            px = pc[:, 0:F]
            py = pc[:, F:2 * F]
            pz = pc[:, 2 * F:3 * F]
            for d in range(3):
                nc.vector.tensor_scalar(pc[:, d * F:(d + 1) * F],
                                        pc[:, d * F:(d + 1) * F],
                                        1e-8, None, Alu.add)
            nc.vector.tensor_copy(pc16[:], pc[:])

            # totals per channel from the PRODUCT pixels (consistency for S4)
            totc = pp.tile([P, 3], f32, tag="totc")
            for d in range(3):
                nc.vector.tensor_reduce(totc[:, d:d + 1],
                                        pc16[:, d * F:(d + 1) * F], X, Alu.add)
            tot3_ps = ps.tile([P, 3], f32, tag="tot3ps")
            nc.tensor.matmul(tot3_ps[:], ones128[:], totc[:], start=True, stop=True)
            nc.vector.tensor_copy(tot3[:], tot3_ps[:])

            nc.scalar.copy(cent_st[0:1, 0:15], cb0[0:1, 0:15])
            rep_ps0 = ps.tile([P, 20], f32, tag="repps0")
            nc.tensor.matmul(rep_ps0[:], onesr[:], cb0[:], start=True, stop=True)
            rep = pp.tile([P, 20], f32, tag="rep0")
            nc.vector.tensor_copy(rep[:], rep_ps0[:])

            prev_traj = None
            for t in range(1, ITERS + 1):
                # ---------- phase 1: scores (identical arithmetic to baseline) ----------
                s_tiles = []
                for k in range(5):
                    u = jnk.tile([P, F], f32, tag=f"u{k}")
                    nc.scalar.activation(
                        u[:], px, Act.Identity,
                        bias=rep[:, 15 + k:16 + k], scale=rep[:, 3 * k:3 * k + 1],
                    )
                    v = jnk.tile([P, F], f32, tag=f"v{k}")
                    s = sco.tile([P, F], f32, tag=f"s{k}")
                    nc.vector.scalar_tensor_tensor(
                        v[:], py, rep[:, 3 * k + 1:3 * k + 2], u[:], Alu.mult, Alu.add)
                    nc.vector.scalar_tensor_tensor(
                        s[:], pz, rep[:, 3 * k + 2:3 * k + 3], v[:], Alu.mult, Alu.add)
                    s_tiles.append(s)

                # previous iteration's trajectory snapshots go behind the u's
                # on ScalarE's queue so they don't delay next-iter scores
                if prev_traj is not None:
                    pcn, pcb, pt = prev_traj
                    nc.scalar.copy(counts_st[0:1, 5 * (pt - 1):5 * pt], pcn[0:1, :])
                    nc.scalar.copy(cent_st[0:1, 15 * pt:15 * (pt + 1)], pcb[0:1, 0:15])

                # ---------- phase 2: max (exact) ----------
                m01 = jnk.tile([P, F], f32, tag="m01")
                m23 = jnk.tile([P, F], f32, tag="m23")
                m = sco.tile([P, F], f32, tag="m")
                nc.vector.tensor_tensor(m01[:], s_tiles[0][:], s_tiles[1][:], Alu.max)
                nc.vector.tensor_tensor(m23[:], s_tiles[2][:], s_tiles[3][:], Alu.max)
                nc.vector.tensor_tensor(m01[:], m01[:], s_tiles[4][:], Alu.max)
                nc.vector.tensor_tensor(m[:], m01[:], m23[:], Alu.max)

                # ---------- phase 3: masks+counts, products, sums ----------
                acc = sm.tile([P, 16], f32, tag="acc")
                mask4 = msk.tile([P, 4 * F], mdt, tag="mask4")
                tots_c = ps.tile([P, 4], f32, tag="tots_c")
                cnts = sm.tile([P, 5], f32, tag="cnts")
                csum = sm.tile([P, 1], f32, tag="csum")
                recip = sm.tile([P, 5], f32, tag="recip")

                if PROD16:
                    # DVE emission is interleaved: non-accumulating products
                    # and the counts chain are spread between accumulating
                    # ops (masks, V-sums) to absorb accumulator-drain stalls
                    prod_fns = []  # (cluster, emit_fn), cluster-sorted
                    for d in range(3):
                        ks = [k for k in range(4) if SUMS[3 * k + d] == "S"]
                        if len(ks) >= 3:
                            def mk_batch(d=d, ks=tuple(ks)):
                                pr = jnk.tile([P, 4 * F], mdt, tag=f"pr4{d}")
                                nc.vector.tensor_tensor(
                                    pr[:].rearrange("p (k f) -> p k f", k=4),
                                    mask4[:].rearrange("p (k f) -> p k f", k=4),
                                    pc16[:, d * F:(d + 1) * F]
                                        .rearrange("p (o f) -> p o f", o=1)
                                        .broadcast_to((P, 4, F)),
                                    Alu.mult)
                                for k in ks:
                                    ja = jnk.tile([P, F], mdt, tag=f"ja{k}{d}")
                                    nc.scalar.activation(
                                        ja[:], pr[:, k * F:(k + 1) * F],
                                        Act.Identity,
                                        accum_out=acc[:, 4 + 3 * k + d:5 + 3 * k + d])
                            prod_fns.append((3, mk_batch))
                        else:
                            for k in ks:
                                def mk_single(d=d, k=k):
                                    pr = jnk.tile([P, F], mdt, tag=f"pr{k}{d}")
                                    nc.vector.tensor_tensor(
                                        pr[:], mask4[:, k * F:(k + 1) * F],
                                        pc16[:, d * F:(d + 1) * F], Alu.mult)
                                    ja = jnk.tile([P, F], mdt, tag=f"ja{k}{d}")
                                    nc.scalar.activation(
                                        ja[:], pr[:], Act.Identity,
                                        accum_out=acc[:, 4 + 3 * k + d:5 + 3 * k + d])
                                prod_fns.append((k, mk_single))
                    prod_fns.sort(key=lambda t: t[0])

                    # masks, with ready products slotted between them
                    for k in range(4):
                        nc.vector.scalar_tensor_tensor(
                            mask4[:, k * F:(k + 1) * F], s_tiles[k][:], 1.0, m[:],
                            Alu.mult, Alu.is_equal, accum_out=acc[:, k:k + 1])
                        if k < 3 and prod_fns and prod_fns[0][0] <= k:
                            prod_fns.pop(0)[1]()

                    # counts all-reduce as soon as the 4 mask accums land
                    nc.tensor.matmul(tots_c[:], ones128[:], acc[:, 0:4],
                                     start=True, stop=True)

                    fillers = [f for _, f in prod_fns]
                    fillers.append(lambda: nc.vector.tensor_copy(cnts[:, 0:4], tots_c[:, 0:4]))
                    fillers.append(lambda: nc.vector.tensor_reduce(csum[:], tots_c[:, 0:4], X, Alu.add))
                    fillers.append(lambda: nc.vector.tensor_scalar(
                        cnts[:, 4:5], csum[:], -1.0, float(N), Alu.mult, Alu.add))
                    fillers.append(lambda: nc.vector.reciprocal(recip[:], cnts[:]))
                    vsums = [(k, d) for k in range(4) for d in range(3)
                             if SUMS[3 * k + d] == "V"]
                    for i, (k, d) in enumerate(vsums):
                        if fillers:
                            fillers.pop(0)()
                        j = jnk.tile([P, F], mdt, tag=f"jv{k}{d}")
                        nc.vector.scalar_tensor_tensor(
                            j[:], pc16[:, d * F:(d + 1) * F], 1.0,
                            mask4[:, k * F:(k + 1) * F], Alu.mult,
                            Alu.mult, accum_out=acc[:, 4 + 3 * k + d:5 + 3 * k + d])
                    for f in fillers:
                        f()
                else:
                    for k in range(4):
                        nc.vector.scalar_tensor_tensor(
                            mask4[:, k * F:(k + 1) * F], s_tiles[k][:], 1.0, m[:],
                            Alu.mult, Alu.is_equal, accum_out=acc[:, k:k + 1])
                    nc.tensor.matmul(tots_c[:], ones128[:], acc[:, 0:4],
                                     start=True, stop=True)
                    nc.vector.tensor_copy(cnts[:, 0:4], tots_c[:, 0:4])
                    nc.vector.tensor_reduce(csum[:], tots_c[:, 0:4], X, Alu.add)
                    nc.vector.tensor_scalar(cnts[:, 4:5], csum[:], -1.0, float(N),
                                            Alu.mult, Alu.add)
                    nc.vector.reciprocal(recip[:], cnts[:])
                    for k in range(4):
                        for d in range(3):
                            col = acc[:, 4 + 3 * k + d:5 + 3 * k + d]
                            pcd = pc[:, d * F:(d + 1) * F]
                            if SUMS[3 * k + d] == "S":
                                pr = jnk.tile([P, F], f32, tag=f"pr{k}{d}")
                                nc.vector.tensor_tensor(
                                    pr[:], mask4[:, k * F:(k + 1) * F], pcd, Alu.mult)
                                ja = jnk.tile([P, F], f32, tag=f"ja{k}{d}")
                                nc.scalar.activation(
                                    ja[:], pr[:], Act.Identity, accum_out=col)
                            else:
                                j = jnk.tile([P, F], f32, tag=f"jv{k}{d}")
                                nc.vector.scalar_tensor_tensor(
                                    j[:], pcd, 1.0, mask4[:, k * F:(k + 1) * F],
                                    Alu.mult, Alu.mult, accum_out=col)

                # ---------- tail (sums all-reduce + center update) ----------
                tots = ps.tile([P, 12], f32, tag="tots")
                nc.tensor.matmul(tots[:], ones128[:], acc[:, 4:16], start=True, stop=True)

                # clusters 0..3 first (centers then biases) so next-iter u_0..3
                # ACTs unblock while the cluster-4 chain is still running
                cb = sm.tile([P, 20], f32, tag="cb")
                sq = sm.tile([P, 15], f32, tag="sq")
                c2 = sm.tile([P, 5], f32, tag="c2")
                nc.vector.tensor_tensor(
                    cb[:, 0:12].rearrange("p (k d) -> p k d", d=3),
                    tots[:, 0:12].rearrange("p (k d) -> p k d", d=3),
                    recip[:, 0:4].rearrange("p (k o) -> p k o", o=1).broadcast_to((P, 4, 3)),
                    Alu.mult)
                nc.vector.tensor_tensor(sq[:, 0:12], cb[:, 0:12], cb[:, 0:12], Alu.mult)
                nc.vector.tensor_reduce(
                    c2[:, 0:4], sq[:, 0:12].rearrange("p (k d) -> p k d", d=3), X, Alu.add)
                nc.vector.tensor_scalar(cb[:, 15:19], c2[:, 0:4], -0.5, 2.0,
                                        Alu.mult, Alu.add)
                s4p = sm.tile([P, 3], f32, tag="s4p")
                nc.vector.tensor_reduce(
                    s4p[:], tots[:, 0:12].rearrange("p (k d) -> p d k", d=3), X, Alu.add)
                S4 = sm.tile([P, 3], f32, tag="S4")
                nc.vector.tensor_tensor(S4[:], tot3[:], s4p[:], Alu.subtract)
                nc.vector.tensor_tensor(
                    cb[:, 12:15],
                    S4[:], recip[:, 4:5].broadcast_to((P, 3)), Alu.mult)
                nc.vector.tensor_tensor(sq[:, 12:15], cb[:, 12:15], cb[:, 12:15], Alu.mult)
                nc.vector.tensor_reduce(
                    c2[:, 4:5], sq[:, 12:15].rearrange("p (k d) -> p k d", d=3), X, Alu.add)
                nc.vector.tensor_scalar(cb[:, 19:20], c2[:, 4:5], -0.5, 2.0,
                                        Alu.mult, Alu.add)

                prev_traj = (cnts, cb, t)
                rep = cb

            # flush final trajectory snapshots
            pcn, pcb, pt = prev_traj
            nc.scalar.copy(counts_st[0:1, 5 * (pt - 1):5 * pt], pcn[0:1, :])
            nc.scalar.copy(cent_st[0:1, 15 * pt:15 * (pt + 1)], pcb[0:1, 0:15])

            nc.sync.dma_start(out=outv[0:1, 0:500], in_=counts_st[:])
            nc.sync.dma_start(out=outv[0:1, 500:OUT_LEN], in_=cent_st[:])
    nc.compile()
    return nc


def _get_nc():
    if "nc" not in _CACHE:
        _CACHE["nc"] = _build_nc()
    return _CACHE["nc"]


def _host_finalize(counts_all, cent_all):
    B = counts_all.shape[0]
    prev = cent_all[:, :-1, :]
    new = cent_all[:, 1:, :]
    with np.errstate(invalid="ignore"):
        ok = np.abs(prev - new) <= np.float32(ATOL) + np.float32(RTOL) * np.abs(new)
    conv_t = np.all(ok, axis=(0, 2))
    idx = np.nonzero(conv_t)[0]
    T = int(idx[0]) + 1 if len(idx) else ITERS + 1
    L = min(T, ITERS)
    centers = cent_all[:, T - 1].reshape(B, K, 3)
    percentages = counts_all[:, L - 1] / np.float32(N)
    centers = np.clip(centers, 0.0, 1.0)
    percentages = np.clip(percentages, 0.0, 1.0)
    color_info = np.concatenate([centers, percentages[..., None]], axis=2).astype(np.float32)
    color_info = np.nan_to_num(color_info, nan=0.0, posinf=1.0, neginf=0.0)
    sort_idx = np.argsort(-color_info[:, :, 3], axis=1, kind="stable")
    return color_info[sort_idx]


def _make_inputs(x, init_idx):
    B = x.shape[0]
    x = np.ascontiguousarray(np.asarray(x, dtype=np.float32))
    init_idx = np.asarray(init_idx).astype(np.int64)
    hh, ww = init_idx // 224, init_idx % 224
    in_maps = []
    for b in range(B):
        c0 = (x[b, :, hh, ww] + np.float32(1e-8)).astype(np.float32)
        cb0 = np.zeros((1, 20), np.float32)
        cb0[0, :15] = c0.reshape(15)
        c2 = (c0 * c0).sum(axis=1, dtype=np.float32)
        cb0[0, 15:20] = np.float32(2.0) - np.float32(0.5) * c2
        in_maps.append({"xp": x[b].reshape(3, N), "cbin": cb0})
    return in_maps


def kernel(x, init_idx):
    from concourse.bass_utils import run_bass_kernel_spmd

    nc = _get_nc()
    in_maps = _make_inputs(x, init_idx)
    res = run_bass_kernel_spmd(nc, in_maps, list(range(8)))
    outs = [np.asarray(r["outv"]).reshape(OUT_LEN) for r in res.results]
    counts_all = np.stack([o[0:500].reshape(100, 5) for o in outs])
    cent_all = np.stack([o[500:OUT_LEN].reshape(101, 15) for o in outs])
    return _host_finalize(counts_all, cent_all)


# revision 4
# speedup vs baseline: 1.1326x; 1.0059x over previous
"""Trainium2 Bass kernel v3 for nn_ColorFeatureExtractor (per-image KMeans).

Data parallel: image b -> core b; 100 Lloyd iterations on-chip; trajectories
streamed back; host picks convergence iteration (same contract as baseline).

v3 vs v2: no gpsimd (it contends with DVE for SBUF ports). fp16 masks and
fp16 channel-major pixel copy; products for all 4 clusters batched into one
2x-rate fp16 TT per channel; sums accumulated on ScalarE ACT (except a few
routed to DVE STT+acc for balance). Counts ride the mask STTs' accumulators
(exact integer sums). Tail on replicated rows.
"""
import os
import sys
import numpy as np

for _p in ("/opt/trn_rl_repo", "/root/.axon_site/_ro/trn_rl_repo"):
    if _p not in sys.path:
        sys.path.append(_p)

K = 5
N = 224 * 224
P = 128
F = N // P             # 392
ITERS = int(os.environ.get("KM_ITERS", "100"))
RTOL, ATOL = 1e-5, 1e-8
OUT_LEN = 500 + 101 * 15

# sums routing: 'S' = ScalarE ACT+acc reading the batched fp16 product,
#               'V' = DVE STT+acc computing product+sum itself (fp16 pixels)
# prod16: batched fp16 products (0 -> fp32 separate TT products, no batching)
_DEF = "sums=SSSSVVSVVSVV;prod16=1"
CFG = os.environ.get("KM_CFG", _DEF)

_CACHE = {}


def _parse_cfg(cfg):
    parts = dict(p.split("=", 1) for p in cfg.split(";") if p)
    sums = parts.get("sums", "S" * 12)
    assert len(sums) == 12 and set(sums) <= set("VS")
    return sums, parts.get("prod16", "1") == "1"


def _build_nc():
    import concourse.bass as bass
    import concourse.mybir as mybir
    from concourse import bacc, tile

    f32 = mybir.dt.float32
    f16 = mybir.dt.float16
    Alu = mybir.AluOpType
    Act = mybir.ActivationFunctionType
    X = mybir.AxisListType.X

    SUMS, PROD16 = _parse_cfg(CFG)
    mdt = f16 if PROD16 else f32

    nc = bacc.Bacc(None, target_bir_lowering=False)
    xp = nc.dram_tensor("xp", [3, N], f32, kind="ExternalInput")
    cbin = nc.dram_tensor("cbin", [1, 20], f32, kind="ExternalInput")
    outv = nc.dram_tensor("outv", [1, OUT_LEN], f32, kind="ExternalOutput")

    with tile.TileContext(nc) as tc:
        with (
            tc.tile_pool(name="persist", bufs=1) as pp,
            tc.tile_pool(name="sco", bufs=2) as sco,
            tc.tile_pool(name="msk", bufs=2) as msk,
            tc.tile_pool(name="jnk", bufs=3) as jnk,
            tc.tile_pool(name="sm", bufs=2) as sm,
            tc.tile_pool(name="ps", bufs=2, space=bass.MemorySpace.PSUM) as ps,
        ):
            # ---- persistent data ----
            pc = pp.tile([P, 3 * F], f32, tag="pc")     # channel-major pixels
            pc16 = pp.tile([P, 3 * F], mdt, tag="pc16")  # product copy
            ones128 = pp.tile([P, 128], f32, tag="ones128")
            onesr = pp.tile([1, 128], f32, tag="onesr")
            tot3 = pp.tile([P, 3], f32, tag="tot3")
            counts_st = pp.tile([1, 500], f32, tag="counts_st")
            cent_st = pp.tile([1, 101 * 15], f32, tag="cent_st")
            cb0 = pp.tile([1, 20], f32, tag="cb0")

            nc.vector.memset(counts_st[:], 0.0)
            nc.vector.memset(cent_st[:], 0.0)
            nc.vector.memset(ones128[:], 1.0)
            nc.vector.memset(onesr[:], 1.0)

            xap = xp[:].rearrange("c (p f) -> c p f", p=P)
            for d in range(3):
                nc.sync.dma_start(out=pc[:, d * F:(d + 1) * F], in_=xap[d])
            nc.sync.dma_start(out=cb0[:], in_=cbin[:])

            px = pc[:, 0:F]
            py = pc[:, F:2 * F]
            pz = pc[:, 2 * F:3 * F]
            for d in range(3):
                nc.vector.tensor_scalar(pc[:, d * F:(d + 1) * F],
                                        pc[:, d * F:(d + 1) * F],
                                        1e-8, None, Alu.add)
            nc.vector.tensor_copy(pc16[:], pc[:])

            # totals per channel from the PRODUCT pixels (consistency for S4)
            totc = pp.tile([P, 3], f32, tag="totc")
            for d in range(3):
                nc.vector.tensor_reduce(totc[:, d:d + 1],
                                        pc16[:, d * F:(d + 1) * F], X, Alu.add)
            tot3_ps = ps.tile([P, 3], f32, tag="tot3ps")
            nc.tensor.matmul(tot3_ps[:], ones128[:], totc[:], start=True, stop=True)
            nc.vector.tensor_copy(tot3[:], tot3_ps[:])

            nc.scalar.copy(cent_st[0:1, 0:15], cb0[0:1, 0:15])
            rep_ps0 = ps.tile([P, 20], f32, tag="repps0")
            nc.tensor.matmul(rep_ps0[:], onesr[:], cb0[:], start=True, stop=True)
            rep = pp.tile([P, 20], f32, tag="rep0")
            nc.vector.tensor_copy(rep[:], rep_ps0[:])

            prev_traj = None
            for t in range(1, ITERS + 1):
                # ---------- phase 1: scores (identical arithmetic to baseline) ----------
                # scores in one contiguous tile so max/masks can batch
                s5 = sco.tile([P, 5 * F], f32, tag="s5")
                s_tiles = [s5[:, k * F:(k + 1) * F] for k in range(5)]
                for k in range(5):
                    u = jnk.tile([P, F], f32, tag=f"u{k}")
                    nc.scalar.activation(
                        u[:], px, Act.Identity,
                        bias=rep[:, 15 + k:16 + k], scale=rep[:, 3 * k:3 * k + 1],
                    )
                    v = jnk.tile([P, F], f32, tag=f"v{k}")
                    nc.vector.scalar_tensor_tensor(
                        v[:], py, rep[:, 3 * k + 1:3 * k + 2], u[:], Alu.mult, Alu.add)
                    nc.vector.scalar_tensor_tensor(
                        s_tiles[k], pz, rep[:, 3 * k + 2:3 * k + 3], v[:], Alu.mult, Alu.add)

                # previous iteration's trajectory snapshots go behind the u's
                # on ScalarE's queue so they don't delay next-iter scores
                if prev_traj is not None:
                    pcn, pcb, pt = prev_traj
                    nc.scalar.copy(counts_st[0:1, 5 * (pt - 1):5 * pt], pcn[0:1, :])
                    nc.scalar.copy(cent_st[0:1, 15 * pt:15 * (pt + 1)], pcb[0:1, 0:15])

                # ---------- phase 2: max (exact under any association) ----------
                mm2 = jnk.tile([P, 2 * F], f32, tag="mm2")
                mC = jnk.tile([P, F], f32, tag="mC")
                m = sco.tile([P, F], f32, tag="m")
                nc.vector.tensor_tensor(mm2[:], s5[:, 0:2 * F], s5[:, 2 * F:4 * F], Alu.max)
                nc.vector.tensor_tensor(mC[:], mm2[:, 0:F], mm2[:, F:2 * F], Alu.max)
                nc.vector.tensor_tensor(m[:], mC[:], s_tiles[4], Alu.max)

                # ---------- phase 3: masks+counts, products, sums ----------
                acc = sm.tile([P, 16], f32, tag="acc")
                mask4 = msk.tile([P, 4 * F], mdt, tag="mask4")
                tots_c = ps.tile([P, 4], f32, tag="tots_c")
                cnts = sm.tile([P, 5], f32, tag="cnts")
                csum = sm.tile([P, 1], f32, tag="csum")
                recip = sm.tile([P, 5], f32, tag="recip")

                if PROD16:
                    # DVE emission is interleaved: non-accumulating products
                    # and the counts chain are spread between accumulating
                    # ops (masks, V-sums) to absorb accumulator-drain stalls
                    prod_fns = []  # (cluster, emit_fn), cluster-sorted
                    singles = {}
                    for d in range(3):
                        ks = [k for k in range(4) if SUMS[3 * k + d] == "S"]
                        if len(ks) >= 3:
                            def mk_batch(d=d, ks=tuple(ks)):
                                pr = jnk.tile([P, 4 * F], mdt, tag=f"pr4{d}")
                                nc.vector.tensor_tensor(
                                    pr[:].rearrange("p (k f) -> p k f", k=4),
                                    mask4[:].rearrange("p (k f) -> p k f", k=4),
                                    pc16[:, d * F:(d + 1) * F]
                                        .rearrange("p (o f) -> p o f", o=1)
                                        .broadcast_to((P, 4, F)),
                                    Alu.mult)
                                for k in ks:
                                    ja = jnk.tile([P, F], mdt, tag=f"ja{k}{d}")
                                    nc.scalar.activation(
                                        ja[:], pr[:, k * F:(k + 1) * F],
                                        Act.Identity,
                                        accum_out=acc[:, 4 + 3 * k + d:5 + 3 * k + d])
                            prod_fns.append((3, mk_batch))
                        else:
                            for k in ks:
                                singles.setdefault(k, []).append(d)
                    for k, ds in singles.items():
                        if ds == [1, 2]:
                            # d1,d2 adjacent in channel-major pc16: one 2F TT
                            def mk_pair(k=k):
                                pr = jnk.tile([P, 2 * F], mdt, tag=f"prp{k}")
                                nc.vector.tensor_tensor(
                                    pr[:].rearrange("p (d f) -> p d f", d=2),
                                    mask4[:, k * F:(k + 1) * F]
                                        .rearrange("p (o f) -> p o f", o=1)
                                        .broadcast_to((P, 2, F)),
                                    pc16[:, F:3 * F].rearrange("p (d f) -> p d f", d=2),
                                    Alu.mult)
                                for i, d in enumerate((1, 2)):
                                    ja = jnk.tile([P, F], mdt, tag=f"ja{k}{d}")
                                    nc.scalar.activation(
                                        ja[:], pr[:, i * F:(i + 1) * F],
                                        Act.Identity,
                                        accum_out=acc[:, 4 + 3 * k + d:5 + 3 * k + d])
                            prod_fns.append((k, mk_pair))
                        else:
                            for d in ds:
                                def mk_single(d=d, k=k):
                                    pr = jnk.tile([P, F], mdt, tag=f"pr{k}{d}")
                                    nc.vector.tensor_tensor(
                                        pr[:], mask4[:, k * F:(k + 1) * F],
                                        pc16[:, d * F:(d + 1) * F], Alu.mult)
                                    ja = jnk.tile([P, F], mdt, tag=f"ja{k}{d}")
                                    nc.scalar.activation(
                                        ja[:], pr[:], Act.Identity,
                                        accum_out=acc[:, 4 + 3 * k + d:5 + 3 * k + d])
                                prod_fns.append((k, mk_single))
                    prod_fns.sort(key=lambda t: t[0])

                    # masks, with ready products slotted between them
                    for k in range(4):
                        nc.vector.scalar_tensor_tensor(
                            mask4[:, k * F:(k + 1) * F], s_tiles[k], 1.0, m[:],
                            Alu.mult, Alu.is_equal, accum_out=acc[:, k:k + 1])
                        if k < 3 and prod_fns and prod_fns[0][0] <= k:
                            prod_fns.pop(0)[1]()

                    # counts all-reduce as soon as the 4 mask accums land
                    nc.tensor.matmul(tots_c[:], ones128[:], acc[:, 0:4],
                                     start=True, stop=True)

                    # remaining products lead their V-sum (ScalarE needs them
                    # early); counts-chain ops trail theirs (their matmul
                    # input lands mid-stream)
                    pre = [f for _, f in prod_fns]
                    post = [
                        lambda: nc.vector.tensor_copy(cnts[:, 0:4], tots_c[:, 0:4]),
                        lambda: nc.vector.tensor_reduce(csum[:], tots_c[:, 0:4], X, Alu.add),
                        lambda: nc.vector.tensor_scalar(
                            cnts[:, 4:5], csum[:], -1.0, float(N), Alu.mult, Alu.add),
                        lambda: nc.vector.reciprocal(recip[:], cnts[:]),
                    ]
                    vsums = [(k, d) for k in range(4) for d in range(3)
                             if SUMS[3 * k + d] == "V"]
                    for i, (k, d) in enumerate(vsums):
                        if pre:
                            pre.pop(0)()
                        j = jnk.tile([P, F], mdt, tag=f"jv{k}{d}")
                        nc.vector.scalar_tensor_tensor(
                            j[:], pc16[:, d * F:(d + 1) * F], 1.0,
                            mask4[:, k * F:(k + 1) * F], Alu.mult,
                            Alu.mult, accum_out=acc[:, 4 + 3 * k + d:5 + 3 * k + d])
                        if not pre and post:
                            post.pop(0)()
                    for f in pre + post:
                        f()
                else:
                    for k in range(4):
                        nc.vector.scalar_tensor_tensor(
                            mask4[:, k * F:(k + 1) * F], s_tiles[k], 1.0, m[:],
                            Alu.mult, Alu.is_equal, accum_out=acc[:, k:k + 1])
                    nc.tensor.matmul(tots_c[:], ones128[:], acc[:, 0:4],
                                     start=True, stop=True)
                    nc.vector.tensor_copy(cnts[:, 0:4], tots_c[:, 0:4])
                    nc.vector.tensor_reduce(csum[:], tots_c[:, 0:4], X, Alu.add)
                    nc.vector.tensor_scalar(cnts[:, 4:5], csum[:], -1.0, float(N),
                                            Alu.mult, Alu.add)
                    nc.vector.reciprocal(recip[:], cnts[:])
                    for k in range(4):
                        for d in range(3):
                            col = acc[:, 4 + 3 * k + d:5 + 3 * k + d]
                            pcd = pc[:, d * F:(d + 1) * F]
                            if SUMS[3 * k + d] == "S":
                                pr = jnk.tile([P, F], f32, tag=f"pr{k}{d}")
                                nc.vector.tensor_tensor(
                                    pr[:], mask4[:, k * F:(k + 1) * F], pcd, Alu.mult)
                                ja = jnk.tile([P, F], f32, tag=f"ja{k}{d}")
                                nc.scalar.activation(
                                    ja[:], pr[:], Act.Identity, accum_out=col)
                            else:
                                j = jnk.tile([P, F], f32, tag=f"jv{k}{d}")
                                nc.vector.scalar_tensor_tensor(
                                    j[:], pcd, 1.0, mask4[:, k * F:(k + 1) * F],
                                    Alu.mult, Alu.mult, accum_out=col)

                # ---------- tail (sums all-reduce + center update) ----------
                tots = ps.tile([P, 12], f32, tag="tots")
                nc.tensor.matmul(tots[:], ones128[:], acc[:, 4:16], start=True, stop=True)

                # clusters 0..3 first (centers then biases) so next-iter u_0..3
                # ACTs unblock while the cluster-4 chain is still running
                cb = sm.tile([P, 20], f32, tag="cb")
                sq = sm.tile([P, 15], f32, tag="sq")
                c2 = sm.tile([P, 5], f32, tag="c2")
                nc.vector.tensor_tensor(
                    cb[:, 0:12].rearrange("p (k d) -> p k d", d=3),
                    tots[:, 0:12].rearrange("p (k d) -> p k d", d=3),
                    recip[:, 0:4].rearrange("p (k o) -> p k o", o=1).broadcast_to((P, 4, 3)),
                    Alu.mult)
                nc.vector.tensor_tensor(sq[:, 0:12], cb[:, 0:12], cb[:, 0:12], Alu.mult)
                nc.vector.tensor_reduce(
                    c2[:, 0:4], sq[:, 0:12].rearrange("p (k d) -> p k d", d=3), X, Alu.add)
                nc.vector.tensor_scalar(cb[:, 15:19], c2[:, 0:4], -0.5, 2.0,
                                        Alu.mult, Alu.add)
                s4p = sm.tile([P, 3], f32, tag="s4p")
                nc.vector.tensor_reduce(
                    s4p[:], tots[:, 0:12].rearrange("p (k d) -> p d k", d=3), X, Alu.add)
                S4 = sm.tile([P, 3], f32, tag="S4")
                nc.vector.tensor_tensor(S4[:], tot3[:], s4p[:], Alu.subtract)
                nc.vector.tensor_tensor(
                    cb[:, 12:15],
                    S4[:], recip[:, 4:5].broadcast_to((P, 3)), Alu.mult)
                nc.vector.tensor_tensor(sq[:, 12:15], cb[:, 12:15], cb[:, 12:15], Alu.mult)
                nc.vector.tensor_reduce(
                    c2[:, 4:5], sq[:, 12:15].rearrange("p (k d) -> p k d", d=3), X, Alu.add)
                nc.vector.tensor_scalar(cb[:, 19:20], c2[:, 4:5], -0.5, 2.0,
                                        Alu.mult, Alu.add)

                prev_traj = (cnts, cb, t)
                rep = cb

            # flush final trajectory snapshots
            pcn, pcb, pt = prev_traj
            nc.scalar.copy(counts_st[0:1, 5 * (pt - 1):5 * pt], pcn[0:1, :])
            nc.scalar.copy(cent_st[0:1, 15 * pt:15 * (pt + 1)], pcb[0:1, 0:15])

            nc.sync.dma_start(out=outv[0:1, 0:500], in_=counts_st[:])
            nc.sync.dma_start(out=outv[0:1, 500:OUT_LEN], in_=cent_st[:])
    nc.compile()
    return nc


def _get_nc():
    if "nc" not in _CACHE:
        _CACHE["nc"] = _build_nc()
    return _CACHE["nc"]


def _host_finalize(counts_all, cent_all):
    B = counts_all.shape[0]
    prev = cent_all[:, :-1, :]
    new = cent_all[:, 1:, :]
    with np.errstate(invalid="ignore"):
        ok = np.abs(prev - new) <= np.float32(ATOL) + np.float32(RTOL) * np.abs(new)
    conv_t = np.all(ok, axis=(0, 2))
    idx = np.nonzero(conv_t)[0]
    T = int(idx[0]) + 1 if len(idx) else ITERS + 1
    L = min(T, ITERS)
    centers = cent_all[:, T - 1].reshape(B, K, 3)
    percentages = counts_all[:, L - 1] / np.float32(N)
    centers = np.clip(centers, 0.0, 1.0)
    percentages = np.clip(percentages, 0.0, 1.0)
    color_info = np.concatenate([centers, percentages[..., None]], axis=2).astype(np.float32)
    color_info = np.nan_to_num(color_info, nan=0.0, posinf=1.0, neginf=0.0)
    sort_idx = np.argsort(-color_info[:, :, 3], axis=1, kind="stable")
    return color_info[sort_idx]


def _make_inputs(x, init_idx):
    B = x.shape[0]
    x = np.ascontiguousarray(np.asarray(x, dtype=np.float32))
    init_idx = np.asarray(init_idx).astype(np.int64)
    hh, ww = init_idx // 224, init_idx % 224
    in_maps = []
    for b in range(B):
        c0 = (x[b, :, hh, ww] + np.float32(1e-8)).astype(np.float32)
        cb0 = np.zeros((1, 20), np.float32)
        cb0[0, :15] = c0.reshape(15)
        c2 = (c0 * c0).sum(axis=1, dtype=np.float32)
        cb0[0, 15:20] = np.float32(2.0) - np.float32(0.5) * c2
        in_maps.append({"xp": x[b].reshape(3, N), "cbin": cb0})
    return in_maps


def kernel(x, init_idx):
    from concourse.bass_utils import run_bass_kernel_spmd

    nc = _get_nc()
    in_maps = _make_inputs(x, init_idx)
    res = run_bass_kernel_spmd(nc, in_maps, list(range(8)))
    outs = [np.asarray(r["outv"]).reshape(OUT_LEN) for r in res.results]
    counts_all = np.stack([o[0:500].reshape(100, 5) for o in outs])
    cent_all = np.stack([o[500:OUT_LEN].reshape(101, 15) for o in outs])
    return _host_finalize(counts_all, cent_all)
